# revision 1
# baseline (speedup 1.0000x reference)
"""DotProductGraphAttention Trainium2 kernel.

Reference computation (per batch b, head h):
    S = Q @ K^T / 8                      [N, N]
    P = softmax(where(adj > 0, S, -inf), axis=-1)
    O = P @ V                            [N, D]
Output: h_prime[B,H,N,D].reshape(N, B, H, D)  (flat reshape)

Softmax is computed max-free (S ~ N(0,1); exp never overflows fp32):
    P = exp(S/8) * A;  O = (P @ V) / rowsum(P)
with the rowsum obtained by augmenting V with a trailing ones column.

Sharding: 8 cores = (batch b in 0..3) x (query half in 0..1). Each core owns
all 8 heads for its (b, 1024-query slice): K/V per head are full [2048, 64],
the adj row-slice [1024, 2048] is shared by all heads on the core.

Per-core pipeline (matmul operands bf16, accumulation fp32):
  - adj:  SWDGE cast-DMA i32->bf16 to HBM scratch, HWDGE transpose-DMA per
          key tile into a resident A^T sbuf tile [128, jt, 1024] (~4 MB).
  - Q,K:  SWDGE cast-DMA f32->bf16 to sbuf; PE transposes build K^T with
          even j-tiles on partitions 0-63 / odd on 64-127 (so QK matmul
          pairs row-tile across array halves) and Q^T replicated on both.
  - S^T:  per slot (j, isup): matmul (d=64 contraction on alternating
          partition halves) into rotating [128, 3, 512] psum window tiles.
          One tile per exp window is essential: Tile tracks PSUM WAR at
          tensor granularity, so a single shared ring tile would serialize
          every QK behind the latest exp.
  - P^T:  ScalarE exp(0.125*S) over one 3-bank window -> bf16 at flat pt
          offsets ((isup, j)-major slot order keeps windows contiguous AND
          lets each query-half's PV start at the head's midpoint);
          VectorE tensor_tensor mult with A^T (bf16 2x mode, same layout).
  - O^T:  PV matmuls with stationary V' = [V|1] (16 LDWEIGHTS per head
          instead of 128 P^T-stationary loads): out [65, 512] psum
          accumulated over j per query half; row 64 is the rowsum. Copied
          to sbuf bf16, PE back-transposed per 128-query tile into
          [128, 4, 66] psum, then reciprocal + broadcast-mult, DMA to HBM.
  - Software pipelining: head h+1's loads/transposes and head h-1's
    PV/normalize are emitted interleaved between head h's QK/exp windows
    (engines execute their streams in FIFO order, so emission distance is
    what absorbs cross-engine handoff latency); PV carries an order-only
    dep on the head's last mask so the scheduler cannot hoist it into the
    head's own window region where it would stall the QK stream.
"""

import sys

if "/opt/trn_rl_repo" not in sys.path:
    sys.path.insert(0, "/opt/trn_rl_repo")

from contextlib import ExitStack

import numpy as np

import concourse.bacc as bacc
import concourse.mybir as mybir
import concourse.tile as tile
from concourse.masks import make_identity
from concourse.tile_rust import add_dep_helper

B, H, N, D = 4, 8, 2048, 64
NCORES = 8
QH = N // 2  # queries per core (1024)
NJT = N // 128  # 16 key tiles
NIT = QH // 128  # 8 query tiles per core
NWIN = 2  # rotating S^T window tiles (WIN psum banks each)
WIN = 3  # banks (slots) per window
BF16 = mybir.dt.bfloat16
F32 = mybir.dt.float32

_CACHED_NC = {}


def build_nc(replay: int = 1):
    """Build + compile the per-core Bass program (same NEFF on all 8 cores)."""
    if replay in _CACHED_NC:
        return _CACHED_NC[replay]

    nc = bacc.Bacc("TRN2", target_bir_lowering=False, debug=False)
    q_h = nc.dram_tensor("q_bh", [H, QH, D], F32, kind="ExternalInput")
    k_h = nc.dram_tensor("k_bh", [H, N, D], F32, kind="ExternalInput")
    v_h = nc.dram_tensor("v_bh", [H, N, D], F32, kind="ExternalInput")
    adj_h = nc.dram_tensor("adj_s", [QH, N], mybir.dt.int32, kind="ExternalInput")
    out_h = nc.dram_tensor("out", [H, QH, D], F32, kind="ExternalOutput")
    adj_scr = nc.dram_tensor("adj_scr", [QH, N], BF16, kind="Internal")

    with tile.TileContext(nc) as tc, ExitStack() as ctx:
        singles = ctx.enter_context(tc.tile_pool(name="singles", bufs=1))
        io = ctx.enter_context(tc.tile_pool(name="io", bufs=3))
        ptp = ctx.enter_context(tc.tile_pool(name="ptp", bufs=3))
        kqp = ctx.enter_context(tc.tile_pool(name="kqp", bufs=3))
        otp = ctx.enter_context(tc.tile_pool(name="otp", bufs=2))
        outp = ctx.enter_context(tc.tile_pool(name="outp", bufs=3))
        ps_ring = ctx.enter_context(tc.tile_pool(name="psring", bufs=NWIN, space="PSUM"))
        ps_ot = ctx.enter_context(tc.tile_pool(name="psot", bufs=1, space="PSUM"))
        ps_tr = ctx.enter_context(tc.tile_pool(name="pstr", bufs=1, space="PSUM"))

        ident = singles.tile([128, 128], BF16)
        make_identity(nc, ident[:])
        at = singles.tile([128, 2, NJT, 512], BF16, tag="at")
        at_flat = at[:].rearrange("p a b c -> p (a b c)")
        last_mask = {"i0": None, "all": None}

        def emit_loads(h):
            kn = io.tile([128, NJT, D], BF16, tag="kn")
            nc.gpsimd.dma_start(
                out=kn[:], in_=k_h[h].rearrange("(j p) d -> p j d", p=128)
            )
            qn = io.tile([128, NIT, D], BF16, tag="qn")
            nc.gpsimd.dma_start(
                out=qn[:], in_=q_h[h].rearrange("(i p) d -> p i d", p=128)
            )
            vp = io.tile([128, NJT, D + 2], BF16, tag="vp")  # 66-wide: 4B-aligned j slices
            nc.vector.memset(vp[:, :, D : D + 1], 1.0)
            nc.gpsimd.dma_start(
                out=vp[:, :, 0:D], in_=v_h[h].rearrange("(j p) d -> p j d", p=128)
            )
            return kn, qn, vp

        def emit_adj_prep():
            """Cast + transpose adj. All DMAs issued up front: the SDMA
            engines pipeline transfers across queues, and masks consume the
            A^T tiles progressively."""
            for c in range(NJT // 4):
                cs = slice(c * 512, (c + 1) * 512)
                nc.gpsimd.dma_start(out=adj_scr[:, cs], in_=adj_h[:, cs])
                for j in range(4 * c, 4 * c + 4):
                    js = slice(j * 128, (j + 1) * 128)
                    for ih in range(2):
                        nc.sync.dma_start(
                            out=at[:, ih, j, :],
                            in_=adj_scr[512 * ih : 512 * (ih + 1), js],
                            transpose=True,
                        )

        def emit_transposes(kn, qn):
            # K^T: one [128,128] transpose per pair of 64-wide K tiles lands
            # even tiles on partitions 0-63 and odd on 64-127.
            kt = kqp.tile([128, NJT // 2, 128], BF16, tag="kt")
            tp = ps_tr.tile([128, 8, 128], BF16, tag="tp")
            for s in range(NJT // 2):
                nc.tensor.transpose(tp[:, s, :], kn[:, 2 * s : 2 * s + 2, :], ident[:])
            nc.vector.tensor_copy(kt[:], tp[:])
            yield
            # Q^T replicated on both partition halves.
            qt = kqp.tile([128, NIT, 128], BF16, tag="qt")
            tq = ps_tr.tile([128, 8, 128], BF16, tag="tp")
            for i in range(NIT):
                nc.tensor.transpose(tq[0:D, i, :], qn[:, i, :], ident[:])
                nc.tensor.transpose(tq[D : 2 * D, i, :], qn[:, i, :], ident[:])
            nc.vector.tensor_copy(qt[:], tq[:])
            yield (kt, qt)

        def emit_windows(h, kt, qt):
            """QK -> exp -> mask in ring windows; yields after each window."""
            pt = ptp.tile([128, NJT * QH], BF16, tag="pt")  # flat [isup, j, 512]
            yield pt
            # slots in (isup outer, j inner) order: each query-half's masks
            # finish by the head's midpoint, so PV of half 0 can overlap the
            # second half's windows. pt/at share the same flat layout.
            slots = [(j, isup) for isup in range(2) for j in range(NJT)]
            for w in range(0, len(slots), WIN):
                width = min(WIN, len(slots) - w)
                # each window gets its own psum tile so the WAR against the
                # window's exp is tracked per-tile (pool rotation = lookahead)
                sp = ps_ring.tile([128, WIN, 512], F32, tag="sring")
                for g, (j, isup) in enumerate(slots[w : w + width]):
                    half = j % 2
                    nc.tensor.matmul(
                        sp[:, g, :],
                        lhsT=kt[64 * half : 64 * half + 64, j // 2, :],
                        rhs=qt[64 * half : 64 * half + 64, 4 * isup : 4 * isup + 4, :],
                        start=True,
                        stop=True,
                    )
                j0, isup0 = slots[w]
                off = (isup0 * NJT + j0) * 512
                nc.scalar.activation(
                    out=pt[:, off : off + width * 512],
                    in_=sp[:, 0:width, :].rearrange("p a b -> p (a b)"),
                    func=mybir.ActivationFunctionType.Exp,
                    scale=0.125,
                )
                tt = nc.vector.tensor_tensor(
                    out=pt[:, off : off + width * 512],
                    in0=pt[:, off : off + width * 512],
                    in1=at_flat[:, off : off + width * 512],
                    op=mybir.AluOpType.mult,
                )
                if any(s == (NJT - 1, 0) for s in slots[w : w + width]):
                    last_mask["i0"] = tt.ins
                last_mask["all"] = tt.ins
                yield

        def emit_pv(h, pt, vp, after_i0, after_all):
            """O^T = V'^T P^T per query half; back-transpose; normalize; store.

            Both halves' matmuls run first (two psum banks), evacuation
            follows. The first matmul of each accumulation group carries an
            order-only dep on the head's last mask so the scheduler cannot
            hoist PV into the head's own window region (where it would stall
            on in-flight masks and head-of-line-block the QK stream).
            """
            ptv = pt.rearrange("p (s j i) -> p s j i", s=2, j=NJT)
            ot_sbs = []
            for ihalf in range(2):
                after_ins = after_i0 if ihalf == 0 else after_all
                ot_ps = ps_ot.tile([65, 512], F32, tag="ot")
                for j in range(NJT):
                    mm = nc.tensor.matmul(
                        ot_ps[:, :],
                        lhsT=vp[:, j, 0 : D + 1],
                        rhs=ptv[:, ihalf, j, :],
                        start=(j == 0),
                        stop=(j == NJT - 1),
                    )
                    if j == 0 and after_ins is not None:
                        add_dep_helper(
                            mm.ins, after_ins, reason="pv after half masks"
                        )
                    if j % 4 == 3:
                        yield
                ot_sb = otp.tile([65, 512], BF16, tag=f"otsb{ihalf}")
                nc.vector.tensor_copy(ot_sb[:], ot_ps[:])
                ot_sbs.append(ot_sb)
                yield
                yield  # emission distance: next psum user waits on this copy
                yield
            for ihalf in range(2):
                ob = ps_ot.tile([128, 4, D + 2], BF16, tag="ot")  # aligned slices
                for itl in range(4):
                    nc.tensor.transpose(
                        ob[:, itl, 0 : D + 1],
                        ot_sbs[ihalf][:, itl * 128 : (itl + 1) * 128],
                        ident[0:65, 0:65],
                    )
                yield
                rr = outp.tile([128, 4, 1], F32, tag="rr")
                nc.vector.reciprocal(out=rr[:], in_=ob[:, :, D : D + 1])
                o_sb = outp.tile([128, 4, D], F32, tag="osb")
                nc.vector.tensor_tensor(
                    out=o_sb[:],
                    in0=ob[:, :, 0:D],
                    in1=rr[:, :, 0:1].to_broadcast([128, 4, D]),
                    op=mybir.AluOpType.mult,
                )
                nc.sync.dma_start(
                    out=out_h[h, 512 * ihalf : 512 * (ihalf + 1), :].rearrange(
                        "(i p) d -> p i d", p=128
                    ),
                    in_=o_sb[:],
                )
                yield

        for rep in range(replay):
            prev_pv = iter(())
            ld = emit_loads(0)
            emit_adj_prep()
            tr = emit_transposes(ld[0], ld[1])
            next(tr)
            kt_qt = next(tr)
            vp = ld[2]
            for h in range(H):
                front = emit_windows(h, *kt_qt)
                pt = next(front)
                nxt_ld = None
                nxt_tr = None
                nxt_kt_qt = None
                nwin = (2 * NJT + WIN - 1) // WIN
                m1, m2, m3 = nwin // 4, nwin // 2, (3 * nwin) // 4
                w = 0
                for _ in front:
                    w += 1
                    next(prev_pv, None)
                    if h + 1 < H:
                        if w == m1:
                            nxt_ld = emit_loads(h + 1)
                        elif w == m2:
                            nxt_tr = emit_transposes(nxt_ld[0], nxt_ld[1])
                            next(nxt_tr)
                        elif w == m3:
                            nxt_kt_qt = next(nxt_tr)
                    next(prev_pv, None)
                for _ in prev_pv:
                    pass
                prev_pv = emit_pv(h, pt, vp, last_mask["i0"], last_mask["all"])
                if h + 1 < H:
                    kt_qt = nxt_kt_qt
                    vp = nxt_ld[2]
            for _ in prev_pv:
                pass

    nc.compile()
    _CACHED_NC[replay] = nc
    return nc


def shard_inputs(queries, keys, values, adj):
    """Per-core input dicts: core c -> (batch c%4, query half c//4)."""
    in_maps = []
    for c in range(NCORES):
        b, qh = c % B, c // B
        in_maps.append(
            {
                "q_bh": np.ascontiguousarray(queries[b, :, qh * QH : (qh + 1) * QH, :]),
                "k_bh": np.ascontiguousarray(keys[b]),
                "v_bh": np.ascontiguousarray(values[b]),
                "adj_s": np.ascontiguousarray(adj[qh * QH : (qh + 1) * QH, :]),
            }
        )
    return in_maps


def assemble_output(results):
    h_prime = np.empty((B, H, N, D), dtype=np.float32)
    for c in range(NCORES):
        b, qh = c % B, c // B
        h_prime[b, :, qh * QH : (qh + 1) * QH, :] = results[c]["out"]
    return h_prime.reshape(N, B, H, D)


def kernel(queries, keys, values, adj):
    queries = np.asarray(queries, dtype=np.float32)
    keys = np.asarray(keys, dtype=np.float32)
    values = np.asarray(values, dtype=np.float32)
    adj = np.asarray(adj, dtype=np.int32)

    from concourse.bass_utils import run_bass_kernel_spmd

    nc = build_nc()
    res = run_bass_kernel_spmd(
        nc, shard_inputs(queries, keys, values, adj), core_ids=list(range(NCORES))
    )
    return assemble_output(res.results)

